# revision 17
# baseline (speedup 1.0000x reference)
"""Trainium2 Bass kernel: single-head attention encoder block (bf16 build).

Problem: x[4, 2048, 1024]; q/k/v projections, softmax attention, output
projection, layernorm.  8 NeuronCores, SPMD.

Sharding: core c handles batch b = c // 2 and query-half h = c % 2.
Each core receives its batch's x ROTATED along the sequence axis so the
core's 1024 query rows occupy rows 0:1024 (attention is permutation-
invariant over keys as long as K and V share an ordering).

All matmul operands are bf16 (the moving-operand dtype gives 1.0
cycles/row on the PE, same as fp32r at >=256 moving rows, but half the
DMA bytes and half the SBUF footprint).  PSUM accumulation stays fp32;
softmax denominators and reciprocals stay fp32.

Score path uses the associativity  S = (x_q Wq)(x Wk)^T = x_q W' x^T
with W' = Wq Wk^T computed on-device: W' (64K cyc) + U^T = W'^T x_q^T
(64K) + weight transposes (16K) replace the K projection (128K) + Q
projection (64K) -- a 48K-cycle/core saving.  The bf16 SBUF budget
keeps BOTH x (natural rows) and x^T resident, so the value path
Z^T = x^T @ exp(S^T) takes its stationary x tiles straight from SBUF.

Per-core dataflow, one uninterrupted PE instruction stream (the TRN2 PE
clock drops to 0.65/1.2 GHz after any idle gap and needs 3us of
continuous work to return to 2.4 GHz, so gaps cost double; warm-up
matmuls with no DMA dependency start the ramp at t=0):

  T_w:   wqT/wkT = Wq^T, Wk^T     (PE transpose via identity matmul)
  W':    W' = Wq Wk^T             ([i, j], from wqT/wkT)
  T_x:   xt = x^T                 (chunks interleaved into W' groups)
  U:     U^T = W'^T x_q^T         ([j, q], from wp + xtb)
  S(qc): S^T = x U^T -> exp via ACT -> at (bf16); den accumulated on PE
         as ones^T @ at, staggered 2 tiles behind the exp drains
  Z(qc): Z^T = x^T @ at           (stationary x tiles from resident xr)
  C2(qc): ctxT = (Wv^T @ Z^T) * recip   (recip = 1/den, fused in drain)
  O(qt): h = ctx @ Wo -> LayerNorm read straight from PSUM
         (bn_stats on PSUM, one ACT Identity op for (h-mu)*rstd via AP
          scale/bias, gamma/beta per 512-half in bf16 2x-mode DVE ops,
          per-half stores), out bf16, host upcasts.

Phase order S0, Z0, S1, C2_0, Z1, O(qt0-3), C2_1, O(qt4-7) keeps every
cross-engine dependency at least one PE group away from its consumer.
"""

from contextlib import ExitStack

import numpy as np

import concourse.bass as bass
import concourse.tile as tile
from concourse import bacc, mybir
from concourse.bass_utils import run_bass_kernel_spmd
from concourse.masks import make_identity

F32 = mybir.dt.float32
BF16 = mybir.dt.bfloat16
AF = mybir.ActivationFunctionType
OP = mybir.AluOpType

B = 4
S = 2048
D = 1024
NQ = 1024     # queries per core
P = 128
DT = D // P   # 8 d-tiles
ST = S // P   # 16 s-tiles
KTN = D // P  # 8 k-tiles
QTN = NQ // P  # 8 q-tiles
NC = 512      # matmul free-dim chunk (one fp32 PSUM bank)
SCN = S // NC   # 4 s-chunks
QCN = NQ // NC  # 2 q-chunks
DCN = D // NC   # 2 d-chunks
N_CORES = 8
SCALE = 1.0 / np.sqrt(np.float32(D))  # 1/32
LN_EPS = 1e-5


def _keepalive(nc, tc, aps, out):
    """Read one column of each AP and DMA to out so bacc keeps the work."""
    kp = tc.alloc_tile_pool(name="keep", bufs=1, side="left")
    n = max(len(aps), 1)
    m = (n + 1) // 2
    kt = kp.tile([P, 2 * m], F32, tag="keep", name="keept")
    for i, ap in enumerate(aps):
        nc.vector.tensor_copy(kt[:, i:i + 1], ap[:, 0:1])
    nc.sync.dma_start(out[0:P, 0:4 * m].bitcast(F32), kt[:])
    kp.release()


def _emit(ctx: ExitStack, tc: tile.TileContext, io: dict, upto: str = "full"):
    nc = tc.nc
    xb = io["xb"]          # [S, D] bf16 (rotated so own queries are rows 0:NQ)
    wq = io["wq"]          # [D, D] bf16
    wk = io["wk"]
    wv = io["wv"]
    wo = io["wo"]
    gamma_b = io["gamma_b"]  # [P, D] bf16 (row-broadcast)
    beta_b = io["beta_b"]
    out = io["out"]        # [NQ, D] bf16

    # ---- constants ----
    const = ctx.enter_context(tc.tile_pool(name="const", bufs=1, side="left"))
    warm = const.tile([P, NC], BF16, tag="warm")
    nc.vector.memset(warm[:], 0.0)
    identity_f = const.tile([P, P], F32, tag="identity_f")
    make_identity(nc, identity_f[:])
    identity = const.tile([P, P], BF16, tag="identity")
    nc.vector.tensor_copy(identity[:], identity_f[:])
    ones = const.tile([P, P], BF16, tag="ones")
    nc.vector.memset(ones[:], 1.0)
    eps_sb = const.tile([P, 1], F32, tag="eps")
    nc.vector.memset(eps_sb[:], LN_EPS)

    # ---- SBUF pools ----
    # left stack (bottom->top): const, xr, wt(wqT+wkT), wkq-rows; wkq is
    # released after the weight transposes, wt after W'; at+ctxT then
    # reuse that space.
    xr_pool = tc.alloc_tile_pool(name="xrp", bufs=1, side="left")
    xr = [xr_pool.tile([P, D], BF16, tag=f"xr{st}", name=f"xr{st}") for st in range(ST)]
    wt_pool = tc.alloc_tile_pool(name="wtp", bufs=1, side="left")
    wqT = [wt_pool.tile([P, D], BF16, tag=f"wqT{k}", name=f"wqT{k}") for k in range(DT)]
    wkT = [wt_pool.tile([P, D], BF16, tag=f"wkT{k}", name=f"wkT{k}") for k in range(DT)]
    wkq_pool = tc.alloc_tile_pool(name="wkqp", bufs=1, side="left")
    wk_sb = [wkq_pool.tile([P, D], BF16, tag=f"wkr{d}", name=f"wkr{d}") for d in range(DT)]
    wq_sb = [wkq_pool.tile([P, D], BF16, tag=f"wqr{d}", name=f"wqr{d}") for d in range(DT)]

    # right stack (bottom->top): gb, recip, zt, ut, xtb, wp; wp released
    # after U, wv+wo allocated in the freed space (their DMA then issues
    # mid-stream, landing long before C2/O need them).
    gb_pool = tc.alloc_tile_pool(name="gbp", bufs=1, side="right")
    gamma_sb = gb_pool.tile([P, D], BF16, tag="gamma", name="gamma_sb")
    beta_sb = gb_pool.tile([P, D], BF16, tag="beta", name="beta_sb")
    recip_pool = tc.alloc_tile_pool(name="recipp", bufs=1, side="right")
    recip = recip_pool.tile([P, NQ], F32, tag="recip", name="recip")
    zt_pool = tc.alloc_tile_pool(name="ztp", bufs=1, side="right")
    zt_sb = [zt_pool.tile([P, NQ], BF16, tag=f"zt{d}", name=f"zt{d}") for d in range(DT)]
    ut_pool = tc.alloc_tile_pool(name="utp", bufs=1, side="right")
    ut_sb = [ut_pool.tile([P, NQ], BF16, tag=f"ut{j}", name=f"ut{j}") for j in range(DT)]
    xtb_pool = tc.alloc_tile_pool(name="xtbp", bufs=1, side="right")
    xtb = [xtb_pool.tile([P, S], BF16, tag=f"xtb{d}", name=f"xtb{d}") for d in range(DT)]
    wp_pool = tc.alloc_tile_pool(name="wpp", bufs=1, side="right")
    wp = [wp_pool.tile([P, D], BF16, tag=f"wp{i}", name=f"wp{i}") for i in range(DT)]

    # ---- DMA issue order (queue is FIFO; arrival order = issue order).
    # Weights first (the weight-transpose + W' front end consumes them),
    # x rows next (T_x chunks interleave into W'), gamma/beta trailing.
    for d in range(DT):
        nc.sync.dma_start(wk_sb[d][:], wk[d * P:(d + 1) * P, :])
    for d in range(DT):
        nc.sync.dma_start(wq_sb[d][:], wq[d * P:(d + 1) * P, :])
    for st in range(ST):
        nc.sync.dma_start(xr[st][:], xb[st * P:(st + 1) * P, :])
    nc.sync.dma_start(gamma_sb[:], gamma_b[:])
    nc.sync.dma_start(beta_sb[:], beta_b[:])

    # ---- PSUM: 7 banks general matmul, 1 bank softmax denominator ----
    ps_mm = ctx.enter_context(tc.tile_pool(name="ps_mm", bufs=7, space="PSUM"))
    ps_den = ctx.enter_context(tc.tile_pool(name="ps_den", bufs=1, space="PSUM"))

    def _drain(i, dst, src):
        """PSUM->SBUF copy, alternating DVE / ACT by index for balance."""
        if i % 2 == 0:
            nc.vector.tensor_copy(dst, src)
        else:
            nc.scalar.copy(dst, src)

    # Warm-up matmuls: no DMA dependency, so they fill the PE pipe while
    # the first weight tiles are still in flight -- and they start the PE
    # busy-streak clock ramping (0.65 -> 1.2 -> 2.4 GHz over 3us of
    # continuous work) so the real matmuls run at full rate.
    wps = ps_mm.tile([P, NC], F32, tag="mm", name="warmps")
    for i in range(8):
        nc.tensor.matmul(wps[:], ones[:], warm[:], start=(i == 0), stop=(i == 7))

    # ---- Phase T_w: wqT/wkT = transposes of the weight row tiles ----
    # One [128,512] PSUM bank packs the transposes of 4 consecutive
    # row-tiles (i) for one k-tile, so each drain is a single wide copy.
    def _tw(rows, dstT, ih):
        for k in range(DT):
            pt = ps_mm.tile([P, NC], F32, tag="mm", name=f"ptW{k}_{ih}")
            for jj in range(4):
                i = 4 * ih + jj
                nc.tensor.matmul(
                    pt[:, jj * P:(jj + 1) * P],
                    rows[i][:, k * P:(k + 1) * P],
                    identity[:],
                    start=True, stop=True,
                )
            _drain(k, dstT[k][:, ih * NC:(ih + 1) * NC], pt[:])

    for ih in range(2):
        _tw(wk_sb, wkT, ih)
    for ih in range(2):
        _tw(wq_sb, wqT, ih)
    wkq_pool.release()

    # ---- Phase T_x: x^T chunk (4 s-tiles, all d) -- called from W' ----
    def _tx_chunk(c):
        for d in range(DT):
            pt = ps_mm.tile([P, NC], F32, tag="mm", name=f"ptT{c}_{d}")
            for j in range(4):
                st = 4 * c + j
                nc.tensor.matmul(
                    pt[:, j * P:(j + 1) * P],
                    xr[st][:, d * P:(d + 1) * P],
                    identity[:],
                    start=True, stop=True,
                )
            _drain(d, xtb[d][:, c * NC:(c + 1) * NC], pt[:])

    # ---- Phase W': W' = Wq Wk^T  ([i, j]), T_x chunks interleaved ----
    txc = 0
    for i in range(DT):
        for jc in range(2):
            g = 2 * i + jc
            if g % 4 == 2 and txc < SCN:
                _tx_chunk(txc)
                txc += 1
            ps = ps_mm.tile([P, NC], F32, tag="mm", name=f"psW{i}_{jc}")
            for k in range(DT):
                nc.tensor.matmul(
                    ps[:],
                    wqT[k][:, i * P:(i + 1) * P],
                    wkT[k][:, jc * NC:(jc + 1) * NC],
                    start=(k == 0),
                    stop=(k == DT - 1),
                )
            _drain(i + jc, wp[i][:, jc * NC:(jc + 1) * NC], ps[:])
    wt_pool.release()
    at_pool = tc.alloc_tile_pool(name="atp", bufs=1, side="left")
    at_sb = [at_pool.tile([P, NQ], BF16, tag=f"at{st}", name=f"at{st}") for st in range(ST)]
    ctxT_pool = tc.alloc_tile_pool(name="ctxTp", bufs=1, side="left")
    ctxT = [ctxT_pool.tile([P, NQ], BF16, tag=f"cxT{v}", name=f"cxT{v}") for v in range(DT)]

    def _release_all(left, right):
        for p in left + right:
            p.release()

    # ---- Phase U: U^T = W'^T x_q^T  ([j, q]) ----
    for j in range(DT):
        for qc in range(QCN):
            if txc < SCN:
                _tx_chunk(txc)
                txc += 1
            ps = ps_mm.tile([P, NC], F32, tag="mm", name=f"psU{j}_{qc}")
            for i in range(DT):
                nc.tensor.matmul(
                    ps[:],
                    wp[i][:, j * P:(j + 1) * P],
                    xtb[i][:, qc * NC:(qc + 1) * NC],
                    start=(i == 0),
                    stop=(i == DT - 1),
                )
            _drain(j + qc, ut_sb[j][:, qc * NC:(qc + 1) * NC], ps[:])
    wp_pool.release()

    # wv/wo allocated in the space wp freed; DMA issues here, landing
    # well before C2 (wv) / O (wo) consume them.
    wv_pool = tc.alloc_tile_pool(name="wvp", bufs=1, side="right")
    wv_sb = [wv_pool.tile([P, D], BF16, tag=f"wv{d}", name=f"wv{d}") for d in range(DT)]
    wo_pool = tc.alloc_tile_pool(name="wop", bufs=1, side="right")
    wo_sb = [wo_pool.tile([P, D], BF16, tag=f"wo{v}", name=f"wo{v}") for v in range(DT)]
    for d in range(DT):
        nc.sync.dma_start(wv_sb[d][:], wv[d * P:(d + 1) * P, :])
    for v in range(DT):
        nc.sync.dma_start(wo_sb[v][:], wo[v * P:(v + 1) * P, :])

    if upto == "U":
        _keepalive(
            nc, tc,
            [t[:, 0:1].bitcast(F32) for t in ut_sb]
            + [t[:, 0:1].bitcast(F32) for t in xtb], out)
        _release_all(
            [ctxT_pool, at_pool, xr_pool],
            [wo_pool, wv_pool, xtb_pool, ut_pool, zt_pool, recip_pool,
             gb_pool])
        return

    # ---- Phase S(qc): scores -> exp (unnormalized), den on PE ----
    den_ps: list = [None, None]

    def _s_phase(qc):
        dps = ps_den.tile([P, NC], F32, tag="den", name=f"dps{qc}")
        den_ps[qc] = dps

        def _den_mm(st):
            nc.tensor.matmul(
                dps[:], ones[:], at_sb[st][:, qc * NC:(qc + 1) * NC],
                start=(st == 0), stop=(st == ST - 1),
            )

        for st in range(ST):
            ps = ps_mm.tile([P, NC], F32, tag="mm", name=f"psS{qc}_{st}")
            for j in range(DT):
                nc.tensor.matmul(
                    ps[:],
                    xtb[j][:, st * P:(st + 1) * P],
                    ut_sb[j][:, qc * NC:(qc + 1) * NC],
                    start=(j == 0),
                    stop=(j == DT - 1),
                )
            # attn = exp(scores / sqrt(dk)); scores are O(1) by
            # construction and softmax is shift-invariant, so no
            # max-subtraction is needed to match the reference.
            nc.scalar.activation(
                at_sb[st][:, qc * NC:(qc + 1) * NC], ps[:], AF.Exp,
                scale=float(SCALE),
            )
            if st >= 2:
                _den_mm(st - 2)
        _den_mm(ST - 2)
        _den_mm(ST - 1)

    def _recip(qc):
        nc.vector.reciprocal(recip[:, qc * NC:(qc + 1) * NC], den_ps[qc][:])

    # ---- Phase Z(qc): Z^T = x^T @ at  (stationary x tiles from SBUF) ----
    def _z_phase(qc):
        for d in range(DT):
            ps = ps_mm.tile([P, NC], F32, tag="mm", name=f"psZ{d}_{qc}")
            for st in range(ST):
                nc.tensor.matmul(
                    ps[:],
                    xr[st][:, d * P:(d + 1) * P],
                    at_sb[st][:, qc * NC:(qc + 1) * NC],
                    start=(st == 0),
                    stop=(st == ST - 1),
                )
            _drain(d, zt_sb[d][:, qc * NC:(qc + 1) * NC], ps[:])

    # ---- Phase C2(qc): ctxT = (Wv^T @ Z^T) * 1/den ----
    def _c2_phase(qc):
        for vt in range(DT):
            ps = ps_mm.tile([P, NC], F32, tag="mm", name=f"psC{vt}_{qc}")
            for d in range(DT):
                nc.tensor.matmul(
                    ps[:],
                    wv_sb[d][:, vt * P:(vt + 1) * P],
                    zt_sb[d][:, qc * NC:(qc + 1) * NC],
                    start=(d == 0),
                    stop=(d == DT - 1),
                )
            nc.vector.tensor_tensor(
                ctxT[vt][:, qc * NC:(qc + 1) * NC],
                ps[:],
                recip[:, qc * NC:(qc + 1) * NC],
                OP.mult,
            )

    # ---- Phase O(qt): h = ctx @ Wo, LayerNorm straight from PSUM ----
    stat_pool = None
    o_pool = None

    def _o_tile(qt):
        pss = []
        for dc in range(DCN):
            ps = ps_mm.tile([P, NC], F32, tag="mm", name=f"psO{qt}_{dc}")
            for v in range(DT):
                nc.tensor.matmul(
                    ps[:],
                    ctxT[v][:, qt * P:(qt + 1) * P],
                    wo_sb[v][:, dc * NC:(dc + 1) * NC],
                    start=(v == 0),
                    stop=(v == DT - 1),
                )
            pss.append(ps)
        stats = stat_pool.tile(
            [P, DCN, nc.vector.BN_STATS_DIM], F32, tag="bnstats", name=f"bnst{qt}"
        )
        for dc in range(DCN):
            nc.vector.bn_stats(out=stats[:, dc, :], in_=pss[dc][:])
        mv = stat_pool.tile([P, nc.vector.BN_AGGR_DIM], F32, tag="bnaggr", name=f"bnag{qt}")
        nc.vector.bn_aggr(out=mv[:], in_=stats[:])
        rstd = stat_pool.tile([P, 1], F32, tag="rstd", name=f"rstd{qt}")
        nc.scalar.activation(rstd[:], mv[:, 1:2], AF.Sqrt, bias=eps_sb[:], scale=1.0)
        nc.vector.reciprocal(rstd[:], rstd[:])
        # -mu*rstd, so the normalize step becomes one ACT op per half:
        # o = Identity(psO*rstd + (-mu*rstd)); gamma/beta per half on DVE
        # (bf16 2x mode) and the store fires per half.
        nmr = stat_pool.tile([P, 1], F32, tag="nmr", name=f"nmr{qt}")
        nc.vector.tensor_scalar(
            out=nmr[:], in0=mv[:, 0:1], scalar1=rstd[:], scalar2=-1.0,
            op0=OP.mult, op1=OP.mult,
        )
        o = o_pool.tile([P, D], BF16, tag="o", name=f"o{qt}")
        for dc in range(DCN):
            sl = slice(dc * NC, (dc + 1) * NC)
            nc.scalar.activation(
                o[:, sl], pss[dc][:], AF.Identity, bias=nmr[:], scale=rstd[:]
            )
            nc.vector.tensor_tensor(o[:, sl], o[:, sl], gamma_sb[:, sl], OP.mult)
            nc.vector.tensor_tensor(o[:, sl], o[:, sl], beta_sb[:, sl], OP.add)
            nc.sync.dma_start(out[qt * P:(qt + 1) * P, sl], o[:, sl])

    # ---- pipelined schedule ----
    _s_phase(0)
    if upto == "S":
        _recip(0)
        _keepalive(
            nc, tc,
            [t[:, 0:1].bitcast(F32) for t in at_sb] + [recip[:, 0:1]], out)
        _release_all(
            [ctxT_pool, at_pool, xr_pool],
            [wo_pool, wv_pool, xtb_pool, ut_pool, zt_pool, recip_pool,
             gb_pool])
        return
    _z_phase(0)
    _recip(0)
    _s_phase(1)
    _c2_phase(0)
    _recip(1)
    _z_phase(1)
    if upto == "C2":
        _keepalive(
            nc, tc,
            [t[:, 0:1].bitcast(F32) for t in ctxT]
            + [t[:, 0:1].bitcast(F32) for t in zt_sb], out)
        _release_all(
            [ctxT_pool, at_pool, xr_pool],
            [wo_pool, wv_pool, xtbb_pool, ut_pool, zt_pool, recip_pool,
             gb_pool])
        return

    stat_pool = tc.alloc_tile_pool(name="statp", bufs=4, side="right")
    o_pool = tc.alloc_tile_pool(name="op", bufs=2, side="right")
    for qt in range(4):
        _o_tile(qt)
    _c2_phase(1)
    for qt in range(4, QTN):
        _o_tile(qt)
    o_pool.release()
    stat_pool.release()
    _release_all(
        [ctxT_pool, at_pool, xr_pool],
        [wo_pool, wv_pool, xtb_pool, ut_pool, zt_pool, recip_pool,
         gb_pool])


_PROGS: dict = {}


def _build_program(n_iters: int = 1, upto: str = "full"):
    key = (n_iters, upto)
    if key not in _PROGS:
        nc = bacc.Bacc(
            "TRN2",
            target_bir_lowering=False,
            debug=False,
            enable_asserts=False,
            num_devices=N_CORES,
        )
        io = {
            "xb": nc.dram_tensor("xb", [S, D], BF16, kind="ExternalInput").ap(),
            "wq": nc.dram_tensor("wq", [D, D], BF16, kind="ExternalInput").ap(),
            "wk": nc.dram_tensor("wk", [D, D], BF16, kind="ExternalInput").ap(),
            "wv": nc.dram_tensor("wv", [D, D], BF16, kind="ExternalInput").ap(),
            "wo": nc.dram_tensor("wo", [D, D], BF16, kind="ExternalInput").ap(),
            "gamma_b": nc.dram_tensor("gamma_b", [P, D], BF16, kind="ExternalInput").ap(),
            "beta_b": nc.dram_tensor("beta_b", [P, D], BF16, kind="ExternalInput").ap(),
            "out": nc.dram_tensor("out", [NQ, D], BF16, kind="ExternalOutput").ap(),
        }
        with tile.TileContext(nc) as tc:
            for _ in range(n_iters):
                with ExitStack() as ctx:
                    _emit(ctx, tc, io, upto)
        nc.compile()
        _PROGS[key] = nc
    return _PROGS[key]


LAST_RESULTS = None


def _bf16(a):
    import ml_dtypes
    return np.ascontiguousarray(np.asarray(a, dtype=np.float32).astype(ml_dtypes.bfloat16))


def kernel(x, Wq, Wk, Wv, Wo, ln2_gamma, ln2_beta):
    global LAST_RESULTS
    x = np.asarray(x, dtype=np.float32)
    wq_b = _bf16(Wq)
    wk_b = _bf16(Wk)
    wv_b = _bf16(Wv)
    wo_b = _bf16(Wo)
    gamma_b = _bf16(np.broadcast_to(np.asarray(ln2_gamma, dtype=np.float32), (P, D)))
    beta_b = _bf16(np.broadcast_to(np.asarray(ln2_beta, dtype=np.float32), (P, D)))

    nc = _build_program()
    in_maps = []
    for c in range(N_CORES):
        b, h = c // 2, c % 2
        # Rotate so this core's query rows are rows 0:NQ.
        xb = _bf16(np.roll(x[b], -h * NQ, axis=0))
        in_maps.append(
            {
                "xb": xb,
                "wq": wq_b,
                "wk": wk_b,
                "wv": wv_b,
                "wo": wo_b,
                "gamma_b": gamma_b,
                "beta_b": beta_b,
            }
        )
    res = run_bass_kernel_spmd(nc, in_maps, list(range(N_CORES)))
    LAST_RESULTS = res
    out = np.empty((B, S, D), dtype=np.float32)
    for c in range(N_CORES):
        b, h = c // 2, c % 2
        out[b, h * NQ:(h + 1) * NQ] = np.asarray(res.results[c]["out"], dtype=np.float32)
    return out


# revision 27
# speedup vs baseline: 1.5139x; 1.5139x over previous
"""Trainium2 Bass kernel: single-head attention encoder block (bf16 build).

Problem: x[4, 2048, 1024]; q/k/v projections, softmax attention, output
projection, layernorm.  8 NeuronCores, SPMD.

Sharding: core c handles batch b = c // 2 and query-half h = c % 2.
Each core receives its batch's x ROTATED along the sequence axis so the
core's 1024 query rows occupy rows 0:1024 (attention is permutation-
invariant over keys as long as K and V share an ordering).

All matmul operands are bf16 (the moving-operand dtype gives 1.0
cycles/row on the PE, same as fp32r at >=256 moving rows, but half the
DMA bytes and half the SBUF footprint).  PSUM accumulation stays fp32;
softmax denominators and reciprocals stay fp32.

Score path uses the associativity  S = (x_q Wq)(x Wk)^T = x_q W' x^T
with W' = Wq Wk^T computed on-device: W' (64K cyc) + U^T = W'^T x_q^T
(64K) + weight transposes (16K) replace the K projection (128K) + Q
projection (64K) -- a 48K-cycle/core saving.  The bf16 SBUF budget
keeps BOTH x (natural rows) and x^T resident, so the value path
Z^T = x^T @ exp(S^T) takes its stationary x tiles straight from SBUF.

Per-core dataflow, one uninterrupted PE instruction stream (the TRN2 PE
clock drops to 0.65/1.2 GHz after any idle gap and needs 3us of
continuous work to return to 2.4 GHz, so gaps cost double; warm-up
matmuls with no DMA dependency start the ramp at t=0):

  T_w:   wqT/wkT = Wq^T, Wk^T     (PE transpose via identity matmul)
  W':    W' = Wq Wk^T             ([i, j], from wqT/wkT)
  T_x:   xt = x^T                 (chunks interleaved into W' groups)
  U:     U^T = W'^T x_q^T         ([j, q], from wp + xtb)
  S(qc): S^T = x U^T -> exp via ACT -> at (bf16); den accumulated on PE
         as ones^T @ at, staggered 2 tiles behind the exp drains
  Z(qc): Z^T = x^T @ at           (stationary x tiles from resident xr)
  C2(qc): ctxT = (Wv^T @ Z^T) * recip   (recip = 1/den, fused in drain)
  O(qt): h = ctx @ Wo -> LayerNorm read straight from PSUM
         (bn_stats on PSUM, one ACT Identity op for (h-mu)*rstd via AP
          scale/bias, gamma/beta per 512-half in bf16 2x-mode DVE ops,
          per-half stores), out bf16, host upcasts.

Phase order S0, Z0, S1, C2_0, Z1, O(qt0-3), C2_1, O(qt4-7) keeps every
cross-engine dependency at least one PE group away from its consumer.
"""

from contextlib import ExitStack

import numpy as np

import concourse.bass as bass
import concourse.tile as tile
from concourse import bacc, mybir
from concourse.bass_utils import run_bass_kernel_spmd
from concourse.masks import make_identity

F32 = mybir.dt.float32
BF16 = mybir.dt.bfloat16
AF = mybir.ActivationFunctionType
OP = mybir.AluOpType

B = 4
S = 2048
D = 1024
NQ = 1024     # queries per core
P = 128
DT = D // P   # 8 d-tiles
ST = S // P   # 16 s-tiles
KTN = D // P  # 8 k-tiles
QTN = NQ // P  # 8 q-tiles
NC = 512      # matmul free-dim chunk (one fp32 PSUM bank)
SCN = S // NC   # 4 s-chunks
QCN = NQ // NC  # 2 q-chunks
DCN = D // NC   # 2 d-chunks
N_CORES = 8
SCALE = 1.0 / np.sqrt(np.float32(D))  # 1/32
LN_EPS = 1e-5


def _keepalive(nc, tc, aps, out):
    """Read one column of each AP and DMA to out so bacc keeps the work."""
    kp = tc.alloc_tile_pool(name="keep", bufs=1, side="left")
    n = max(len(aps), 1)
    m = (n + 1) // 2
    kt = kp.tile([P, 2 * m], F32, tag="keep", name="keept")
    for i, ap in enumerate(aps):
        nc.vector.tensor_copy(kt[:, i:i + 1], ap[:, 0:1])
    nc.sync.dma_start(out[0:P, 0:4 * m].bitcast(F32), kt[:])
    kp.release()


def _emit(ctx: ExitStack, tc: tile.TileContext, io: dict, upto: str = "full"):
    nc = tc.nc
    xb = io["xb"]          # [S, D] bf16 (rotated so own queries are rows 0:NQ)
    wq = io["wq"]          # [D, D] bf16
    wk = io["wk"]
    wv = io["wv"]
    wo = io["wo"]
    gamma_b = io["gamma_b"]  # [P, D] bf16 (row-broadcast)
    beta_b = io["beta_b"]
    out = io["out"]        # [NQ, D] bf16

    # ---- constants ----
    const = ctx.enter_context(tc.tile_pool(name="const", bufs=1, side="left"))
    warm = const.tile([P, P], BF16, tag="warm")
    nc.vector.memset(warm[:], 0.0)
    identity_f = const.tile([P, P], F32, tag="identity_f")
    make_identity(nc, identity_f[:])
    identity = const.tile([P, P], BF16, tag="identity")
    nc.vector.tensor_copy(identity[:], identity_f[:])
    ones = const.tile([P, P], BF16, tag="ones")
    nc.vector.memset(ones[:], 1.0)
    eps_sb = const.tile([P, 1], F32, tag="eps")
    nc.vector.memset(eps_sb[:], LN_EPS)

    # ---- SBUF pools ----
    # left stack (bottom->top): const, xr, wt(wqT+wkT), wkq-rows; wkq is
    # released after the weight transposes, wt after W'; at+ctxT then
    # reuse that space.
    xr_pool = tc.alloc_tile_pool(name="xrp", bufs=1, side="left")
    xr = [xr_pool.tile([P, D], BF16, tag=f"xr{st}", name=f"xr{st}") for st in range(ST)]
    wt_pool = tc.alloc_tile_pool(name="wtp", bufs=1, side="left")
    wqT = [wt_pool.tile([P, D], BF16, tag=f"wqT{k}", name=f"wqT{k}") for k in range(DT)]
    wkT = [wt_pool.tile([P, D], BF16, tag=f"wkT{k}", name=f"wkT{k}") for k in range(DT)]
    wkq_pool = tc.alloc_tile_pool(name="wkqp", bufs=1, side="left")
    wk_sb = [wkq_pool.tile([P, D], BF16, tag=f"wkr{d}", name=f"wkr{d}") for d in range(DT)]
    wq_sb = [wkq_pool.tile([P, D], BF16, tag=f"wqr{d}", name=f"wqr{d}") for d in range(DT)]

    # right stack (bottom->top): gb, recip, zt, ut, xtb, wp; wp released
    # after U, wv+wo allocated in the freed space (their DMA then issues
    # mid-stream, landing long before C2/O need them).
    gb_pool = tc.alloc_tile_pool(name="gbp", bufs=1, side="right")
    gamma_sb = gb_pool.tile([P, D], BF16, tag="gamma", name="gamma_sb")
    beta_sb = gb_pool.tile([P, D], BF16, tag="beta", name="beta_sb")
    recip_pool = tc.alloc_tile_pool(name="recipp", bufs=1, side="right")
    recip = recip_pool.tile([P, NQ], F32, tag="recip", name="recip")
    zt_pool = tc.alloc_tile_pool(name="ztp", bufs=1, side="right")
    zt_sb = [zt_pool.tile([P, NQ], BF16, tag=f"zt{d}", name=f"zt{d}") for d in range(DT)]
    ut_pool = tc.alloc_tile_pool(name="utp", bufs=1, side="right")
    ut_sb = [ut_pool.tile([P, NQ], BF16, tag=f"ut{j}", name=f"ut{j}") for j in range(DT)]
    xtb_pool = tc.alloc_tile_pool(name="xtbp", bufs=1, side="right")
    xtb = [xtb_pool.tile([P, S], BF16, tag=f"xtb{d}", name=f"xtb{d}") for d in range(DT)]
    wp_pool = tc.alloc_tile_pool(name="wpp", bufs=1, side="right")
    wp = [wp_pool.tile([P, D], BF16, tag=f"wp{i}", name=f"wp{i}") for i in range(DT)]

    # ---- DMA issue order (queue is FIFO; arrival order = issue order).
    # Weights first (the weight-transpose + W' front end consumes them),
    # x rows next (T_x chunks interleave into W'), gamma/beta trailing.
    for d in range(0, 4):
        nc.sync.dma_start(wk_sb[d][:], wk[d * P:(d + 1) * P, :])
    for d in range(0, 4):
        nc.sync.dma_start(wq_sb[d][:], wq[d * P:(d + 1) * P, :])
    for d in range(4, 8):
        nc.sync.dma_start(wk_sb[d][:], wk[d * P:(d + 1) * P, :])
    for d in range(4, 8):
        nc.sync.dma_start(wq_sb[d][:], wq[d * P:(d + 1) * P, :])
    for st in range(ST):
        nc.sync.dma_start(xr[st][:], xb[st * P:(st + 1) * P, :])
    nc.sync.dma_start(gamma_sb[:], gamma_b[:])
    nc.sync.dma_start(beta_sb[:], beta_b[:])

    # ---- PSUM: 7 banks general matmul, 1 bank softmax denominator ----
    ps_mm = ctx.enter_context(tc.tile_pool(name="ps_mm", bufs=7, space="PSUM"))
    ps_den = ctx.enter_context(tc.tile_pool(name="ps_den", bufs=1, space="PSUM"))

    def _drain(i, dst, src):
        """PSUM->SBUF copy, alternating DVE / ACT by index for balance."""
        if i % 2 == 0:
            nc.vector.tensor_copy(dst, src)
        else:
            nc.scalar.copy(dst, src)

    # Warm-up / pad matmuls: no DMA dependency (warm is the very first
    # DVE memset), so they fill the PE pipe while DMA is still in flight
    # -- and they keep the PE busy-streak clock ramped (0.65 -> 1.2 ->
    # 2.4 GHz over 3us of continuous work; ANY idle gap resets it).
    _padn = [0]

    def _pad(n):
        wps = ps_mm.tile([P, NC], F32, tag="mm", name=f"pad{_padn[0]}")
        _padn[0] += 1
        for i in range(n):
            nc.tensor.matmul(
                wps[:, 0:P], warm[:, 0:P], warm[:, 0:P],
                start=(i == 0), stop=(i == n - 1))

    for _ in range(4):
        _pad(8)

    # ---- Phase T_w: wqT/wkT = transposes of the weight row tiles ----
    # One [128,512] PSUM bank packs the transposes of 4 consecutive
    # row-tiles (i) for one k-tile, so each drain is a single wide copy.
    def _tw(rows, dstT, ih):
        for k in range(DT):
            pt = ps_mm.tile([P, NC], F32, tag="mm", name=f"ptW{k}_{ih}")
            for jj in range(4):
                i = 4 * ih + jj
                nc.tensor.matmul(
                    pt[:, jj * P:(jj + 1) * P],
                    rows[i][:, k * P:(k + 1) * P],
                    identity[:],
                    start=True, stop=True,
                )
            _drain(k, dstT[k][:, ih * NC:(ih + 1) * NC], pt[:])

    _tw(wk_sb, wkT, 0)
    _pad(6)
    _tw(wq_sb, wqT, 0)
    _pad(6)
    wkq_release = [False]

    # ---- Phase T_x: x^T chunk (4 s-tiles, all d) -- called from W' ----
    def _tx_chunk(c):
        for d in range(DT):
            pt = ps_mm.tile([P, NC], F32, tag="mm", name=f"ptT{c}_{d}")
            for j in range(4):
                st = 4 * c + j
                nc.tensor.matmul(
                    pt[:, j * P:(j + 1) * P],
                    xr[st][:, d * P:(d + 1) * P],
                    identity[:],
                    start=True, stop=True,
                )
            _drain(d, xtb[d][:, c * NC:(c + 1) * NC], pt[:])

    # ---- Phase W': W' = Wq Wk^T  ([i, j]) ----
    # Group (i, jc) needs wqT half i//4 and wkT half jc, so the first
    # four groups run as soon as the FIRST halves of wk/wq have landed
    # and been transposed; the remaining T_w halves and the T_x chunks
    # interleave into the W' stream as their DMA lands.
    def _wp_group(i, jc):
        ps = ps_mm.tile([P, NC], F32, tag="mm", name=f"psW{i}_{jc}")
        for k in range(DT):
            nc.tensor.matmul(
                ps[:],
                wqT[k][:, i * P:(i + 1) * P],
                wkT[k][:, jc * NC:(jc + 1) * NC],
                start=(k == 0),
                stop=(k == DT - 1),
            )
        _drain(i + jc, wp[i][:, jc * NC:(jc + 1) * NC], ps[:])

    txc = 0
    for i in range(0, 4):
        _wp_group(i, 0)
    _tw(wk_sb, wkT, 1)
    for i in range(0, 4):
        _wp_group(i, 1)
    _tw(wq_sb, wqT, 1)
    wkq_pool.release()
    for i in range(4, 8):
        for jc in range(2):
            g = 2 * i + jc
            if g % 3 == 0 and txc < SCN:
                _tx_chunk(txc)
                txc += 1
            _wp_group(i, jc)
    wt_pool.release()
    at_pool = tc.alloc_tile_pool(name="atp", bufs=1, side="left")
    at_sb = [at_pool.tile([P, NQ], BF16, tag=f"at{st}", name=f"at{st}") for st in range(ST)]
    ctxT_pool = tc.alloc_tile_pool(name="ctxTp", bufs=1, side="left")
    ctxT = [ctxT_pool.tile([P, NQ], BF16, tag=f"cxT{v}", name=f"cxT{v}") for v in range(DT)]

    def _release_all(left, right):
        for p in left + right:
            p.release()

    # ---- Phase U: U^T = W'^T x_q^T  ([j, q]) ----
    for j in range(DT):
        for qc in range(QCN):
            if txc < SCN:
                _tx_chunk(txc)
                txc += 1
            ps = ps_mm.tile([P, NC], F32, tag="mm", name=f"psU{j}_{qc}")
            for i in range(DT):
                nc.tensor.matmul(
                    ps[:],
                    wp[i][:, j * P:(j + 1) * P],
                    xtb[i][:, qc * NC:(qc + 1) * NC],
                    start=(i == 0),
                    stop=(i == DT - 1),
                )
            _drain(j + qc, ut_sb[j][:, qc * NC:(qc + 1) * NC], ps[:])
    wp_pool.release()

    # wv/wo allocated in the space wp freed; DMA issues here, landing
    # well before C2 (wv) / O (wo) consume them.
    wv_pool = tc.alloc_tile_pool(name="wvp", bufs=1, side="right")
    wv_sb = [wv_pool.tile([P, D], BF16, tag=f"wv{d}", name=f"wv{d}") for d in range(DT)]
    wo_pool = tc.alloc_tile_pool(name="wop", bufs=1, side="right")
    wo_sb = [wo_pool.tile([P, D], BF16, tag=f"wo{v}", name=f"wo{v}") for v in range(DT)]
    for d in range(DT):
        nc.sync.dma_start(wv_sb[d][:], wv[d * P:(d + 1) * P, :])
    for v in range(DT):
        nc.sync.dma_start(wo_sb[v][:], wo[v * P:(v + 1) * P, :])

    if upto == "U":
        _keepalive(
            nc, tc,
            [t[:, 0:1].bitcast(F32) for t in ut_sb]
            + [t[:, 0:1].bitcast(F32) for t in xtb], out)
        _release_all(
            [ctxT_pool, at_pool, xr_pool],
            [wo_pool, wv_pool, xtb_pool, ut_pool, zt_pool, recip_pool,
             gb_pool])
        return

    # ---- Phase S(qc): scores -> exp (unnormalized), den on PE ----
    den_ps: list = [None, None]

    def _s_phase(qc):
        dps = ps_den.tile([P, NC], F32, tag="den", name=f"dps{qc}")
        den_ps[qc] = dps

        def _den_mm(st):
            nc.tensor.matmul(
                dps[:], ones[:], at_sb[st][:, qc * NC:(qc + 1) * NC],
                start=(st == 0), stop=(st == ST - 1),
            )

        for st in range(ST):
            ps = ps_mm.tile([P, NC], F32, tag="mm", name=f"psS{qc}_{st}")
            for j in range(DT):
                nc.tensor.matmul(
                    ps[:],
                    xtb[j][:, st * P:(st + 1) * P],
                    ut_sb[j][:, qc * NC:(qc + 1) * NC],
                    start=(j == 0),
                    stop=(j == DT - 1),
                )
            # attn = exp(scores / sqrt(dk)); scores are O(1) by
            # construction and softmax is shift-invariant, so no
            # max-subtraction is needed to match the reference.
            nc.scalar.activation(
                at_sb[st][:, qc * NC:(qc + 1) * NC], ps[:], AF.Exp,
                scale=float(SCALE),
            )
            if st >= 2:
                _den_mm(st - 2)
        _den_mm(ST - 2)
        _den_mm(ST - 1)

    def _recip(qc):
        nc.vector.reciprocal(recip[:, qc * NC:(qc + 1) * NC], den_ps[qc][:])

    # ---- Phase Z(qc): Z^T = x^T @ at  (stationary x tiles from SBUF) ----
    def _z_phase(qc):
        for d in range(DT):
            ps = ps_mm.tile([P, NC], F32, tag="mm", name=f"psZ{d}_{qc}")
            for st in range(ST):
                nc.tensor.matmul(
                    ps[:],
                    xr[st][:, d * P:(d + 1) * P],
                    at_sb[st][:, qc * NC:(qc + 1) * NC],
                    start=(st == 0),
                    stop=(st == ST - 1),
                )
            _drain(d, zt_sb[d][:, qc * NC:(qc + 1) * NC], ps[:])

    # ---- Phase C2(qc): ctxT = (Wv^T @ Z^T) * 1/den ----
    def _c2_phase(qc):
        for vt in range(DT):
            ps = ps_mm.tile([P, NC], F32, tag="mm", name=f"psC{vt}_{qc}")
            for d in range(DT):
                nc.tensor.matmul(
                    ps[:],
                    wv_sb[d][:, vt * P:(vt + 1) * P],
                    zt_sb[d][:, qc * NC:(qc + 1) * NC],
                    start=(d == 0),
                    stop=(d == DT - 1),
                )
            nc.vector.tensor_tensor(
                ctxT[vt][:, qc * NC:(qc + 1) * NC],
                ps[:],
                recip[:, qc * NC:(qc + 1) * NC],
                OP.mult,
            )

    # ---- Phase O(qt): h = ctx @ Wo, LayerNorm straight from PSUM ----
    stat_pool = None
    o_pool = None

    def _o_tile(qt):
        fin = _o_part(qt)
        fin()

    def _o_part(qt):
        pss = []
        for dc in range(DCN):
            ps = ps_mm.tile([P, NC], F32, tag="mm", name=f"psO{qt}_{dc}")
            for v in range(DT):
                nc.tensor.matmul(
                    ps[:],
                    ctxT[v][:, qt * P:(qt + 1) * P],
                    wo_sb[v][:, dc * NC:(dc + 1) * NC],
                    start=(v == 0),
                    stop=(v == DT - 1),
                )
            pss.append(ps)
        stats = stat_pool.tile(
            [P, DCN, nc.vector.BN_STATS_DIM], F32, tag="bnstats", name=f"bnst{qt}"
        )
        for dc in range(DCN):
            nc.vector.bn_stats(out=stats[:, dc, :], in_=pss[dc][:])
        mv = stat_pool.tile([P, nc.vector.BN_AGGR_DIM], F32, tag="bnaggr", name=f"bnag{qt}")
        nc.vector.bn_aggr(out=mv[:], in_=stats[:])
        rstd = stat_pool.tile([P, 1], F32, tag="rstd", name=f"rstd{qt}")
        nc.scalar.activation(rstd[:], mv[:, 1:2], AF.Sqrt, bias=eps_sb[:], scale=1.0)
        nc.vector.reciprocal(rstd[:], rstd[:])
        # -mu*rstd, so the normalize step becomes one ACT op per half:
        # o = Identity(psO*rstd + (-mu*rstd)); gamma/beta per half on DVE
        # (bf16 2x mode) and the store fires per half.
        nmr = stat_pool.tile([P, 1], F32, tag="nmr", name=f"nmr{qt}")
        nc.vector.tensor_scalar(
            out=nmr[:], in0=mv[:, 0:1], scalar1=rstd[:], scalar2=-1.0,
            op0=OP.mult, op1=OP.mult,
        )
        o = o_pool.tile([P, D], BF16, tag="o", name=f"o{qt}")

        def _finish():
            for dc in range(DCN):
                sl = slice(dc * NC, (dc + 1) * NC)
                nc.scalar.activation(
                    o[:, sl], pss[dc][:], AF.Identity, bias=nmr[:], scale=rstd[:]
                )
                nc.vector.tensor_tensor(o[:, sl], o[:, sl], gamma_sb[:, sl], OP.mult)
                nc.vector.tensor_tensor(o[:, sl], o[:, sl], beta_sb[:, sl], OP.add)
                nc.sync.dma_start(out[qt * P:(qt + 1) * P, sl], o[:, sl])
        return _finish

    # ---- pipelined schedule ----
    _s_phase(0)
    if upto == "S":
        _recip(0)
        _keepalive(
            nc, tc,
            [t[:, 0:1].bitcast(F32) for t in at_sb] + [recip[:, 0:1]], out)
        _release_all(
            [ctxT_pool, at_pool, xr_pool],
            [wo_pool, wv_pool, xtb_pool, ut_pool, zt_pool, recip_pool,
             gb_pool])
        return
    _z_phase(0)
    _recip(0)
    _s_phase(1)
    _c2_phase(0)
    _recip(1)
    _z_phase(1)
    if upto == "C2":
        _keepalive(
            nc, tc,
            [t[:, 0:1].bitcast(F32) for t in ctxT]
            + [t[:, 0:1].bitcast(F32) for t in zt_sb], out)
        _release_all(
            [ctxT_pool, at_pool, xr_pool],
            [wo_pool, wv_pool, xtb_pool, ut_pool, zt_pool, recip_pool,
             gb_pool])
        return

    stat_pool = tc.alloc_tile_pool(name="statp", bufs=4, side="right")
    o_pool = tc.alloc_tile_pool(name="op", bufs=3, side="right")
    for qt in range(4):
        _o_tile(qt)
    _c2_phase(1)
    _o_tile(4)
    _o_tile(5)
    fin6 = _o_part(6)
    fin7 = _o_part(7)
    fin6()
    fin7()
    o_pool.release()
    stat_pool.release()
    _release_all(
        [ctxT_pool, at_pool, xr_pool],
        [wo_pool, wv_pool, xtb_pool, ut_pool, zt_pool, recip_pool,
         gb_pool])


_PROGS: dict = {}


def _build_program(n_iters: int = 1, upto: str = "full"):
    key = (n_iters, upto)
    if key not in _PROGS:
        nc = bacc.Bacc(
            "TRN2",
            target_bir_lowering=False,
            debug=False,
            enable_asserts=False,
            num_devices=N_CORES,
        )
        io = {
            "xb": nc.dram_tensor("xb", [S, D], BF16, kind="ExternalInput").ap(),
            "wq": nc.dram_tensor("wq", [D, D], BF16, kind="ExternalInput").ap(),
            "wk": nc.dram_tensor("wk", [D, D], BF16, kind="ExternalInput").ap(),
            "wv": nc.dram_tensor("wv", [D, D], BF16, kind="ExternalInput").ap(),
            "wo": nc.dram_tensor("wo", [D, D], BF16, kind="ExternalInput").ap(),
            "gamma_b": nc.dram_tensor("gamma_b", [P, D], BF16, kind="ExternalInput").ap(),
            "beta_b": nc.dram_tensor("beta_b", [P, D], BF16, kind="ExternalInput").ap(),
            "out": nc.dram_tensor("out", [NQ, D], BF16, kind="ExternalOutput").ap(),
        }
        with tile.TileContext(nc) as tc:
            for _ in range(n_iters):
                with ExitStack() as ctx:
                    _emit(ctx, tc, io, upto)
        nc.compile()
        _PROGS[key] = nc
    return _PROGS[key]


LAST_RESULTS = None


def _bf16(a):
    import ml_dtypes
    return np.ascontiguousarray(np.asarray(a, dtype=np.float32).astype(ml_dtypes.bfloat16))


def kernel(x, Wq, Wk, Wv, Wo, ln2_gamma, ln2_beta):
    global LAST_RESULTS
    x = np.asarray(x, dtype=np.float32)
    wq_b = _bf16(Wq)
    wk_b = _bf16(Wk)
    wv_b = _bf16(Wv)
    wo_b = _bf16(Wo)
    gamma_b = _bf16(np.broadcast_to(np.asarray(ln2_gamma, dtype=np.float32), (P, D)))
    beta_b = _bf16(np.broadcast_to(np.asarray(ln2_beta, dtype=np.float32), (P, D)))

    nc = _build_program()
    in_maps = []
    for c in range(N_CORES):
        b, h = c // 2, c % 2
        # Rotate so this core's query rows are rows 0:NQ.
        xb = _bf16(np.roll(x[b], -h * NQ, axis=0))
        in_maps.append(
            {
                "xb": xb,
                "wq": wq_b,
                "wk": wk_b,
                "wv": wv_b,
                "wo": wo_b,
                "gamma_b": gamma_b,
                "beta_b": beta_b,
            }
        )
    res = run_bass_kernel_spmd(nc, in_maps, list(range(N_CORES)))
    LAST_RESULTS = res
    out = np.empty((B, S, D), dtype=np.float32)
    for c in range(N_CORES):
        b, h = c // 2, c % 2
        out[b, h * NQ:(h + 1) * NQ] = np.asarray(res.results[c]["out"], dtype=np.float32)
    return out


# revision 30
# speedup vs baseline: 2.1323x; 1.4085x over previous
"""Trainium2 Bass kernel: single-head attention encoder block (bf16 build).

Problem: x[4, 2048, 1024]; q/k/v projections, softmax attention, output
projection, layernorm.  8 NeuronCores, SPMD.

Sharding: core c handles batch b = c // 2 and query-half h = c % 2.
Each core receives its batch's x ROTATED along the sequence axis so the
core's 1024 query rows occupy rows 0:1024 (attention is permutation-
invariant over keys as long as K and V share an ordering).

All matmul operands are bf16 (the moving-operand dtype gives 1.0
cycles/row on the PE, same as fp32r at >=256 moving rows, but half the
DMA bytes and half the SBUF footprint).  PSUM accumulation stays fp32;
softmax denominators and reciprocals stay fp32.

Score path uses the associativity  S = (x_q Wq)(x Wk)^T = x_q W' x^T
with W' = Wq Wk^T computed on-device: W' (64K cyc) + U^T = W'^T x_q^T
(64K) + weight transposes (16K) replace the K projection (128K) + Q
projection (64K) -- a 48K-cycle/core saving.  The bf16 SBUF budget
keeps BOTH x (natural rows) and x^T resident, so the value path
Z^T = x^T @ exp(S^T) takes its stationary x tiles straight from SBUF.

Per-core dataflow, one uninterrupted PE instruction stream (the TRN2 PE
clock drops to 0.65/1.2 GHz after any idle gap and needs 3us of
continuous work to return to 2.4 GHz, so gaps cost double; warm-up
matmuls with no DMA dependency start the ramp at t=0):

  T_w:   wqT/wkT = Wq^T, Wk^T     (PE transpose via identity matmul)
  W':    W' = Wq Wk^T             ([i, j], from wqT/wkT)
  T_x:   xt = x^T                 (chunks interleaved into W' groups)
  U:     U^T = W'^T x_q^T         ([j, q], from wp + xtb)
  S(qc): S^T = x U^T -> exp via ACT -> at (bf16); den accumulated on PE
         as ones^T @ at, staggered 2 tiles behind the exp drains
  Z(qc): Z^T = x^T @ at           (stationary x tiles from resident xr)
  C2(qc): ctxT = (Wv^T @ Z^T) * recip   (recip = 1/den, fused in drain)
  O(qt): h = ctx @ Wo -> LayerNorm read straight from PSUM
         (bn_stats on PSUM, one ACT Identity op for (h-mu)*rstd via AP
          scale/bias, gamma/beta per 512-half in bf16 2x-mode DVE ops,
          per-half stores), out bf16, host upcasts.

Phase order S0, Z0, S1, C2_0, Z1, O(qt0-3), C2_1, O(qt4-7) keeps every
cross-engine dependency at least one PE group away from its consumer.
"""

from contextlib import ExitStack

import numpy as np

import concourse.bass as bass
import concourse.tile as tile
from concourse import bacc, mybir
from concourse.bass_utils import run_bass_kernel_spmd
from concourse.masks import make_identity

F32 = mybir.dt.float32
BF16 = mybir.dt.bfloat16
AF = mybir.ActivationFunctionType
OP = mybir.AluOpType

B = 4
S = 2048
D = 1024
NQ = 1024     # queries per core
P = 128
DT = D // P   # 8 d-tiles
ST = S // P   # 16 s-tiles
KTN = D // P  # 8 k-tiles
QTN = NQ // P  # 8 q-tiles
NC = 512      # matmul free-dim chunk (one fp32 PSUM bank)
SCN = S // NC   # 4 s-chunks
QCN = NQ // NC  # 2 q-chunks
DCN = D // NC   # 2 d-chunks
N_CORES = 8
SCALE = 1.0 / np.sqrt(np.float32(D))  # 1/32
LN_EPS = 1e-5


def _keepalive(nc, tc, aps, out):
    """Read one column of each AP and DMA to out so bacc keeps the work."""
    kp = tc.alloc_tile_pool(name="keep", bufs=1, side="left")
    n = max(len(aps), 1)
    m = (n + 1) // 2
    kt = kp.tile([P, 2 * m], F32, tag="keep", name="keept")
    for i, ap in enumerate(aps):
        nc.vector.tensor_copy(kt[:, i:i + 1], ap[:, 0:1])
    nc.sync.dma_start(out[0:P, 0:4 * m].bitcast(F32), kt[:])
    kp.release()


def _emit(ctx: ExitStack, tc: tile.TileContext, io: dict, upto: str = "full",
          first: bool = True):
    nc = tc.nc
    xb = io["xb"]          # [S, D] bf16 (rotated so own queries are rows 0:NQ)
    wq = io["wq"]          # [D, D] bf16
    wk = io["wk"]
    wv = io["wv"]
    wo = io["wo"]
    gamma_b = io["gamma_b"]  # [P, D] bf16 (row-broadcast)
    beta_b = io["beta_b"]
    out = io["out"]        # [NQ, D] bf16

    # ---- constants ----
    const = ctx.enter_context(tc.tile_pool(name="const", bufs=1, side="left"))
    warm = const.tile([P, P], BF16, tag="warm")
    nc.vector.memset(warm[:], 0.0)
    identity_f = const.tile([P, P], F32, tag="identity_f")
    make_identity(nc, identity_f[:])
    identity = const.tile([P, P], BF16, tag="identity")
    nc.vector.tensor_copy(identity[:], identity_f[:])
    ones = const.tile([P, P], BF16, tag="ones")
    nc.vector.memset(ones[:], 1.0)
    eps_sb = const.tile([P, 1], F32, tag="eps")
    nc.vector.memset(eps_sb[:], LN_EPS)

    # ---- SBUF pools ----
    # left stack (bottom->top): const, xr, wt(wqT+wkT), wkq-rows; wkq is
    # released after the weight transposes, wt after W'; at+ctxT then
    # reuse that space.
    xr_pool = tc.alloc_tile_pool(name="xrp", bufs=1, side="left")
    xr = [xr_pool.tile([P, D], BF16, tag=f"xr{st}", name=f"xr{st}") for st in range(ST)]
    wt_pool = tc.alloc_tile_pool(name="wtp", bufs=1, side="left")
    wqT = [wt_pool.tile([P, D], BF16, tag=f"wqT{k}", name=f"wqT{k}") for k in range(DT)]
    wkT = [wt_pool.tile([P, D], BF16, tag=f"wkT{k}", name=f"wkT{k}") for k in range(DT)]
    wkq_pool = tc.alloc_tile_pool(name="wkqp", bufs=1, side="left")
    wk_sb = [wkq_pool.tile([P, D], BF16, tag=f"wkr{d}", name=f"wkr{d}") for d in range(DT)]
    wq_sb = [wkq_pool.tile([P, D], BF16, tag=f"wqr{d}", name=f"wqr{d}") for d in range(DT)]

    # right stack (bottom->top): gb, recip, zt, ut, xtb, wp; wp released
    # after U, wv+wo allocated in the freed space (their DMA then issues
    # mid-stream, landing long before C2/O need them).
    gb_pool = tc.alloc_tile_pool(name="gbp", bufs=1, side="right")
    gamma_sb = gb_pool.tile([P, D], BF16, tag="gamma", name="gamma_sb")
    beta_sb = gb_pool.tile([P, D], BF16, tag="beta", name="beta_sb")
    recip_pool = tc.alloc_tile_pool(name="recipp", bufs=1, side="right")
    recip = recip_pool.tile([P, NQ], F32, tag="recip", name="recip")
    zt_pool = tc.alloc_tile_pool(name="ztp", bufs=1, side="right")
    zt_sb = [zt_pool.tile([P, NQ], BF16, tag=f"zt{d}", name=f"zt{d}") for d in range(DT)]
    ut_pool = tc.alloc_tile_pool(name="utp", bufs=1, side="right")
    ut_sb = [ut_pool.tile([P, NQ], BF16, tag=f"ut{j}", name=f"ut{j}") for j in range(DT)]
    xtb_pool = tc.alloc_tile_pool(name="xtbp", bufs=1, side="right")
    xtb = [xtb_pool.tile([P, S], BF16, tag=f"xtb{d}", name=f"xtb{d}") for d in range(DT)]
    wp_pool = tc.alloc_tile_pool(name="wpp", bufs=1, side="right")
    wp = [wp_pool.tile([P, D], BF16, tag=f"wp{i}", name=f"wp{i}") for i in range(DT)]

    # ---- DMA issue order (queue is FIFO; arrival order = issue order).
    # Weights first (the weight-transpose + W' front end consumes them),
    # x rows next (T_x chunks interleave into W'), gamma/beta trailing.
    for d in range(0, 4):
        nc.sync.dma_start(wk_sb[d][:], wk[d * P:(d + 1) * P, :])
    for d in range(0, 4):
        nc.sync.dma_start(wq_sb[d][:], wq[d * P:(d + 1) * P, :])
    for d in range(4, 8):
        nc.sync.dma_start(wk_sb[d][:], wk[d * P:(d + 1) * P, :])
    for d in range(4, 8):
        nc.sync.dma_start(wq_sb[d][:], wq[d * P:(d + 1) * P, :])
    for st in range(ST):
        nc.sync.dma_start(xr[st][:], xb[st * P:(st + 1) * P, :])
    nc.sync.dma_start(gamma_sb[:], gamma_b[:])
    nc.sync.dma_start(beta_sb[:], beta_b[:])

    # ---- PSUM: 7 banks general matmul, 1 bank softmax denominator ----
    ps_mm = ctx.enter_context(tc.tile_pool(name="ps_mm", bufs=7, space="PSUM"))
    ps_den = ctx.enter_context(tc.tile_pool(name="ps_den", bufs=1, space="PSUM"))

    def _drain(i, dst, src):
        """PSUM->SBUF copy, alternating DVE / ACT by index for balance."""
        if i % 2 == 0:
            nc.vector.tensor_copy(dst, src)
        else:
            nc.scalar.copy(dst, src)

    # Warm-up / pad matmuls: no DMA dependency (warm is the very first
    # DVE memset), so they fill the PE pipe while DMA is still in flight
    # -- and they keep the PE busy-streak clock ramped (0.65 -> 1.2 ->
    # 2.4 GHz over 3us of continuous work; ANY idle gap resets it).
    _padn = [0]

    def _pad(n):
        wps = ps_mm.tile([P, NC], F32, tag="mm", name=f"pad{_padn[0]}")
        _padn[0] += 1
        for i in range(n):
            nc.tensor.matmul(
                wps[:, 0:P], warm[:, 0:P], warm[:, 0:P],
                start=(i == 0), stop=(i == n - 1))

    if first:
        for _ in range(4):
            _pad(8)

    # ---- Phase T_w: wqT/wkT = transposes of the weight row tiles ----
    # One [128,512] PSUM bank packs the transposes of 4 consecutive
    # row-tiles (i) for one k-tile, so each drain is a single wide copy.
    def _tw(rows, dstT, ih):
        for k in range(DT):
            pt = ps_mm.tile([P, NC], F32, tag="mm", name=f"ptW{k}_{ih}")
            for jj in range(4):
                i = 4 * ih + jj
                nc.tensor.matmul(
                    pt[:, jj * P:(jj + 1) * P],
                    rows[i][:, k * P:(k + 1) * P],
                    identity[:],
                    start=True, stop=True,
                )
            _drain(k, dstT[k][:, ih * NC:(ih + 1) * NC], pt[:])

    _tw(wk_sb, wkT, 0)
    if first:
        _pad(6)
    _tw(wq_sb, wqT, 0)
    if first:
        _pad(6)

    # ---- Phase T_x: x^T chunk (4 s-tiles, all d) -- called from W' ----
    def _tx_chunk(c):
        for d in range(DT):
            pt = ps_mm.tile([P, NC], F32, tag="mm", name=f"ptT{c}_{d}")
            for j in range(4):
                st = 4 * c + j
                nc.tensor.matmul(
                    pt[:, j * P:(j + 1) * P],
                    xr[st][:, d * P:(d + 1) * P],
                    identity[:],
                    start=True, stop=True,
                )
            _drain(d, xtb[d][:, c * NC:(c + 1) * NC], pt[:])

    # ---- Phase W': W' = Wq Wk^T  ([i, j]) ----
    # Group (i, jc) needs wqT half i//4 and wkT half jc, so the first
    # four groups run as soon as the FIRST halves of wk/wq have landed
    # and been transposed; the remaining T_w halves and the T_x chunks
    # interleave into the W' stream as their DMA lands.
    def _wp_group(i, jc):
        ps = ps_mm.tile([P, NC], F32, tag="mm", name=f"psW{i}_{jc}")
        for k in range(DT):
            nc.tensor.matmul(
                ps[:],
                wqT[k][:, i * P:(i + 1) * P],
                wkT[k][:, jc * NC:(jc + 1) * NC],
                start=(k == 0),
                stop=(k == DT - 1),
            )
        _drain(i + jc, wp[i][:, jc * NC:(jc + 1) * NC], ps[:])

    txc = 0
    for i in range(0, 4):
        _wp_group(i, 0)
    _tw(wk_sb, wkT, 1)
    for i in range(0, 4):
        _wp_group(i, 1)
    _tw(wq_sb, wqT, 1)
    wkq_pool.release()
    for i in range(4, 8):
        for jc in range(2):
            g = 2 * i + jc
            if g % 3 == 0 and txc < SCN:
                _tx_chunk(txc)
                txc += 1
            _wp_group(i, jc)
    wt_pool.release()
    at_pool = tc.alloc_tile_pool(name="atp", bufs=1, side="left")
    at_sb = [at_pool.tile([P, NQ], BF16, tag=f"at{st}", name=f"at{st}") for st in range(ST)]
    ctxT_pool = tc.alloc_tile_pool(name="ctxTp", bufs=1, side="left")
    ctxT = [ctxT_pool.tile([P, NQ], BF16, tag=f"cxT{v}", name=f"cxT{v}") for v in range(DT)]

    def _release_all(left, right):
        for p in left + right:
            p.release()

    # ---- Phase U: U^T = W'^T x_q^T  ([j, q]) ----
    for j in range(DT):
        for qc in range(QCN):
            if txc < SCN:
                _tx_chunk(txc)
                txc += 1
            ps = ps_mm.tile([P, NC], F32, tag="mm", name=f"psU{j}_{qc}")
            for i in range(DT):
                nc.tensor.matmul(
                    ps[:],
                    wp[i][:, j * P:(j + 1) * P],
                    xtb[i][:, qc * NC:(qc + 1) * NC],
                    start=(i == 0),
                    stop=(i == DT - 1),
                )
            _drain(j + qc, ut_sb[j][:, qc * NC:(qc + 1) * NC], ps[:])
    wp_pool.release()

    # wv/wo allocated in the space wp freed; DMA issues here, landing
    # well before C2 (wv) / O (wo) consume them.
    wv_pool = tc.alloc_tile_pool(name="wvp", bufs=1, side="right")
    wv_sb = [wv_pool.tile([P, D], BF16, tag=f"wv{d}", name=f"wv{d}") for d in range(DT)]
    wo_pool = tc.alloc_tile_pool(name="wop", bufs=1, side="right")
    wo_sb = [wo_pool.tile([P, D], BF16, tag=f"wo{v}", name=f"wo{v}") for v in range(DT)]
    for d in range(DT):
        nc.sync.dma_start(wv_sb[d][:], wv[d * P:(d + 1) * P, :])
    for v in range(DT):
        nc.sync.dma_start(wo_sb[v][:], wo[v * P:(v + 1) * P, :])

    if upto == "U":
        _keepalive(
            nc, tc,
            [t[:, 0:1].bitcast(F32) for t in ut_sb]
            + [t[:, 0:1].bitcast(F32) for t in xtb], out)
        _release_all(
            [ctxT_pool, at_pool, xr_pool],
            [wo_pool, wv_pool, xtb_pool, ut_pool, zt_pool, recip_pool,
             gb_pool])
        return

    # ---- Phase S(qc): scores -> exp (unnormalized), den on PE ----
    den_ps: list = [None, None]

    def _s_phase(qc):
        dps = ps_den.tile([P, NC], F32, tag="den", name=f"dps{qc}")
        den_ps[qc] = dps

        def _den_mm(st):
            nc.tensor.matmul(
                dps[:], ones[:], at_sb[st][:, qc * NC:(qc + 1) * NC],
                start=(st == 0), stop=(st == ST - 1),
            )

        for st in range(ST):
            ps = ps_mm.tile([P, NC], F32, tag="mm", name=f"psS{qc}_{st}")
            for j in range(DT):
                nc.tensor.matmul(
                    ps[:],
                    xtb[j][:, st * P:(st + 1) * P],
                    ut_sb[j][:, qc * NC:(qc + 1) * NC],
                    start=(j == 0),
                    stop=(j == DT - 1),
                )
            # attn = exp(scores / sqrt(dk)); scores are O(1) by
            # construction and softmax is shift-invariant, so no
            # max-subtraction is needed to match the reference.
            nc.scalar.activation(
                at_sb[st][:, qc * NC:(qc + 1) * NC], ps[:], AF.Exp,
                scale=float(SCALE),
            )
            if st >= 2:
                _den_mm(st - 2)
        _den_mm(ST - 2)
        _den_mm(ST - 1)

    def _recip(qc):
        nc.vector.reciprocal(recip[:, qc * NC:(qc + 1) * NC], den_ps[qc][:])

    # ---- Phase Z(qc): Z^T = x^T @ at  (stationary x tiles from SBUF) ----
    def _z_phase(qc):
        for d in range(DT):
            ps = ps_mm.tile([P, NC], F32, tag="mm", name=f"psZ{d}_{qc}")
            for st in range(ST):
                nc.tensor.matmul(
                    ps[:],
                    xr[st][:, d * P:(d + 1) * P],
                    at_sb[st][:, qc * NC:(qc + 1) * NC],
                    start=(st == 0),
                    stop=(st == ST - 1),
                )
            _drain(d, zt_sb[d][:, qc * NC:(qc + 1) * NC], ps[:])

    # ---- Phase C2(qc): ctxT = (Wv^T @ Z^T) * 1/den ----
    def _c2_phase(qc):
        for vt in range(DT):
            ps = ps_mm.tile([P, NC], F32, tag="mm", name=f"psC{vt}_{qc}")
            for d in range(DT):
                nc.tensor.matmul(
                    ps[:],
                    wv_sb[d][:, vt * P:(vt + 1) * P],
                    zt_sb[d][:, qc * NC:(qc + 1) * NC],
                    start=(d == 0),
                    stop=(d == DT - 1),
                )
            nc.vector.tensor_tensor(
                ctxT[vt][:, qc * NC:(qc + 1) * NC],
                ps[:],
                recip[:, qc * NC:(qc + 1) * NC],
                OP.mult,
            )

    # ---- Phase O(qt): h = ctx @ Wo, LayerNorm straight from PSUM ----
    stat_pool = None
    o_pool = None

    def _o_tile(qt):
        fin = _o_part(qt)
        fin()

    def _o_part(qt):
        pss = []
        for dc in range(DCN):
            ps = ps_mm.tile([P, NC], F32, tag="mm", name=f"psO{qt}_{dc}")
            for v in range(DT):
                nc.tensor.matmul(
                    ps[:],
                    ctxT[v][:, qt * P:(qt + 1) * P],
                    wo_sb[v][:, dc * NC:(dc + 1) * NC],
                    start=(v == 0),
                    stop=(v == DT - 1),
                )
            pss.append(ps)
        stats = stat_pool.tile(
            [P, DCN, nc.vector.BN_STATS_DIM], F32, tag="bnstats", name=f"bnst{qt}"
        )
        for dc in range(DCN):
            nc.vector.bn_stats(out=stats[:, dc, :], in_=pss[dc][:])
        mv = stat_pool.tile([P, nc.vector.BN_AGGR_DIM], F32, tag="bnaggr", name=f"bnag{qt}")
        nc.vector.bn_aggr(out=mv[:], in_=stats[:])
        rstd = stat_pool.tile([P, 1], F32, tag="rstd", name=f"rstd{qt}")
        nc.scalar.activation(rstd[:], mv[:, 1:2], AF.Sqrt, bias=eps_sb[:], scale=1.0)
        nc.vector.reciprocal(rstd[:], rstd[:])
        # -mu*rstd, so the normalize step becomes one ACT op per half:
        # o = Identity(psO*rstd + (-mu*rstd)); gamma/beta per half on DVE
        # (bf16 2x mode) and the store fires per half.
        nmr = stat_pool.tile([P, 1], F32, tag="nmr", name=f"nmr{qt}")
        nc.vector.tensor_scalar(
            out=nmr[:], in0=mv[:, 0:1], scalar1=rstd[:], scalar2=-1.0,
            op0=OP.mult, op1=OP.mult,
        )
        o = o_pool.tile([P, D], BF16, tag="o", name=f"o{qt}")

        def _finish():
            for dc in range(DCN):
                sl = slice(dc * NC, (dc + 1) * NC)
                nc.scalar.activation(
                    o[:, sl], pss[dc][:], AF.Identity, bias=nmr[:], scale=rstd[:]
                )
                nc.vector.tensor_tensor(o[:, sl], o[:, sl], gamma_sb[:, sl], OP.mult)
                nc.vector.tensor_tensor(o[:, sl], o[:, sl], beta_sb[:, sl], OP.add)
                nc.sync.dma_start(out[qt * P:(qt + 1) * P, sl], o[:, sl])
        return _finish

    # ---- pipelined schedule ----
    _s_phase(0)
    if upto == "S":
        _recip(0)
        _keepalive(
            nc, tc,
            [t[:, 0:1].bitcast(F32) for t in at_sb] + [recip[:, 0:1]], out)
        _release_all(
            [ctxT_pool, at_pool, xr_pool],
            [wo_pool, wv_pool, xtb_pool, ut_pool, zt_pool, recip_pool,
             gb_pool])
        return
    _z_phase(0)
    _recip(0)
    _s_phase(1)
    _c2_phase(0)
    _recip(1)
    _z_phase(1)
    if upto == "C2":
        _keepalive(
            nc, tc,
            [t[:, 0:1].bitcast(F32) for t in ctxT]
            + [t[:, 0:1].bitcast(F32) for t in zt_sb], out)
        _release_all(
            [ctxT_pool, at_pool, xr_pool],
            [wo_pool, wv_pool, xtb_pool, ut_pool, zt_pool, recip_pool,
             gb_pool])
        return

    stat_pool = tc.alloc_tile_pool(name="statp", bufs=4, side="right")
    o_pool = tc.alloc_tile_pool(name="op", bufs=3, side="right")
    for qt in range(4):
        _o_tile(qt)
    _c2_phase(1)
    _o_tile(4)
    _o_tile(5)
    fin6 = _o_part(6)
    fin7 = _o_part(7)
    fin6()
    fin7()
    o_pool.release()
    stat_pool.release()
    _release_all(
        [ctxT_pool, at_pool, xr_pool],
        [wo_pool, wv_pool, xtb_pool, ut_pool, zt_pool, recip_pool,
         gb_pool])


_PROGS: dict = {}


def _build_program(n_iters: int = 1, upto: str = "full"):
    key = (n_iters, upto)
    if key not in _PROGS:
        nc = bacc.Bacc(
            "TRN2",
            target_bir_lowering=False,
            debug=False,
            enable_asserts=False,
            num_devices=N_CORES,
        )
        io = {
            "xb": nc.dram_tensor("xb", [S, D], BF16, kind="ExternalInput").ap(),
            "wq": nc.dram_tensor("wq", [D, D], BF16, kind="ExternalInput").ap(),
            "wk": nc.dram_tensor("wk", [D, D], BF16, kind="ExternalInput").ap(),
            "wv": nc.dram_tensor("wv", [D, D], BF16, kind="ExternalInput").ap(),
            "wo": nc.dram_tensor("wo", [D, D], BF16, kind="ExternalInput").ap(),
            "gamma_b": nc.dram_tensor("gamma_b", [P, D], BF16, kind="ExternalInput").ap(),
            "beta_b": nc.dram_tensor("beta_b", [P, D], BF16, kind="ExternalInput").ap(),
            "out": nc.dram_tensor("out", [NQ, D], BF16, kind="ExternalOutput").ap(),
        }
        with tile.TileContext(nc) as tc:
            for it in range(n_iters):
                with ExitStack() as ctx:
                    _emit(ctx, tc, io, upto, first=(it == 0))
        nc.compile()
        _PROGS[key] = nc
    return _PROGS[key]


LAST_RESULTS = None


def _bf16(a):
    import ml_dtypes
    return np.ascontiguousarray(np.asarray(a, dtype=np.float32).astype(ml_dtypes.bfloat16))


def kernel(x, Wq, Wk, Wv, Wo, ln2_gamma, ln2_beta):
    global LAST_RESULTS
    x = np.asarray(x, dtype=np.float32)
    wq_b = _bf16(Wq)
    wk_b = _bf16(Wk)
    wv_b = _bf16(Wv)
    wo_b = _bf16(Wo)
    gamma_b = _bf16(np.broadcast_to(np.asarray(ln2_gamma, dtype=np.float32), (P, D)))
    beta_b = _bf16(np.broadcast_to(np.asarray(ln2_beta, dtype=np.float32), (P, D)))

    nc = _build_program()
    in_maps = []
    for c in range(N_CORES):
        b, h = c // 2, c % 2
        # Rotate so this core's query rows are rows 0:NQ.
        xb = _bf16(np.roll(x[b], -h * NQ, axis=0))
        in_maps.append(
            {
                "xb": xb,
                "wq": wq_b,
                "wk": wk_b,
                "wv": wv_b,
                "wo": wo_b,
                "gamma_b": gamma_b,
                "beta_b": beta_b,
            }
        )
    res = run_bass_kernel_spmd(nc, in_maps, list(range(N_CORES)))
    LAST_RESULTS = res
    out = np.empty((B, S, D), dtype=np.float32)
    for c in range(N_CORES):
        b, h = c // 2, c % 2
        out[b, h * NQ:(h + 1) * NQ] = np.asarray(res.results[c]["out"], dtype=np.float32)
    return out
